# revision 19
# baseline (speedup 1.0000x reference)
"""Trainium2 Bass kernel for nn_ChatDecoder: greedy LSTM decoder, 32 steps.

Strategy (8 NeuronCores, SPMD):
  - Vocab-sharded dense projection: each core holds its W_dense[:, c*4000:(c+1)*4000]
    slice resident in SBUF (as an fp16 two-term split: W1=fp16(W),
    W2=fp16((W-W1)*2^11)) and computes logits [64, 4000] per step with
    fp32-quality accuracy at fp16 matmul speed:
      logits = 2^-11 * (A1s@W1 + A2@W1 + A1@W2),  A1=fp16(h), A1s=A1*2^11,
      A2=fp16((h-A1)*2^11) — all three accumulate in one PSUM bank.
  - Unit-sharded LSTM: each core computes 128 of the 1024 hidden units
    (all four gates, batch-major [64, 128] tiles), then an AllGather
    assembles full h.T on every core (8 per-block DMAs back to SBUF).
  - The x @ W_ih + b path is folded into a host-precomputed (float64) table
    videmb = emb @ W_ih[:, own-cols] + b[own-cols]  [32000, 512] per core;
    each step indirect-DMA-gathers videmb[idx] -> zx, so only h @ W_hh runs
    on the PE (8 wide fp32 matmuls, emitted early so they overlap the
    previous step's argmax/AllGather/gather chain).
  - Greedy argmax: two-half DVE max/max_index (first half hidden under the
    dense), tiny AllGather of (value, global index), local combine with
    first-occurrence tie-break matching jnp.argmax.
  - Gates use tanh only: sigmoid(x) = 0.5 + 0.5*tanh(x/2) (tanh's table is
    ~10x more accurate than sigmoid's; greedy decode here has argmax
    margins down to 1.4e-6, so everything on the argmax path is kept at
    fp32-or-better accuracy).

Output per core: [32, 64, 4000] (step, batch, vocab shard); host concatenates
shards and transposes to [64, 32, 32000].
"""

import sys
from contextlib import ExitStack

import numpy as np

for _p in ("/opt/trn_rl_repo",):
    if _p not in sys.path:
        sys.path.insert(0, _p)

import concourse.bass as bass
import concourse.tile as tile
from concourse import bacc, mybir
from concourse.bass_utils import run_bass_kernel_spmd

F32 = mybir.dt.float32
F16 = mybir.dt.float16
I32 = mybir.dt.int32
U32 = mybir.dt.uint32
TANH = mybir.ActivationFunctionType.Tanh
OP = mybir.AluOpType

V, E, U, B, T_FULL = 32000, 512, 1024, 64, 32
NC = 8
VS = V // NC          # 4000 vocab shard
NT = 500              # dense moving tile (<=512)
NTILES = VS // NT     # 8
KD = U // 128         # 8 dense K-chunks
GO = 1
RG = [list(range(NC))]
SC = 2048.0           # 2^11 split scale


def build_program(T: int = T_FULL, has_bd: bool = False):
    nc = bacc.Bacc(
        "TRN2", target_bir_lowering=False, debug=False, num_devices=NC
    )

    def inp(name, shape, dtype=F32):
        return nc.dram_tensor(name, list(shape), dtype, kind="ExternalInput")

    h0t = inp("h0t", (128, 8 * 64))
    c0 = inp("c0", (64, 128))
    videmb_d = inp("videmb", (V, 512))
    zx0_d = inp("zx0", (64, 512))
    whh_d = inp("whh", (128, 8 * 512))
    wd1_d = inp("wd1", (128, KD * VS), F16)
    wd2_d = inp("wd2", (128, KD * VS), F16)
    offs_d = inp("offs8", (64, NTILES))
    id_d = inp("ident", (64, 64))
    if has_bd:
        bd_d = inp("bd", (64, VS))
    out_d = nc.dram_tensor("out", [T, B, VS], F32, kind="ExternalOutput")

    with tile.TileContext(nc) as tc, ExitStack() as ctx:
        const = ctx.enter_context(tc.tile_pool(name="const", bufs=1))
        hpool = ctx.enter_context(tc.tile_pool(name="hpool", bufs=2))
        spool = ctx.enter_context(tc.tile_pool(name="spool", bufs=2))
        cpool = ctx.enter_context(tc.tile_pool(name="cpool", bufs=2))
        zxpool = ctx.enter_context(tc.tile_pool(name="zxpool", bufs=2))
        gates = ctx.enter_context(tc.tile_pool(name="gates", bufs=2))
        lpool = ctx.enter_context(tc.tile_pool(name="lpool", bufs=1))
        ampool = ctx.enter_context(tc.tile_pool(name="ampool", bufs=2))
        dram = ctx.enter_context(tc.tile_pool(name="dram", bufs=2, space="DRAM"))
        zpsum = ctx.enter_context(tc.tile_pool(name="zpsum", bufs=1, space="PSUM"))
        dpsum = ctx.enter_context(tc.tile_pool(name="dpsum", bufs=3, space="PSUM"))
        tpsum = ctx.enter_context(tc.tile_pool(name="tpsum", bufs=2, space="PSUM"))

        whh = const.tile([128, 8 * 512], F32)
        nc.sync.dma_start(whh[:], whh_d[:])
        wd1 = const.tile([128, KD * VS], F16)
        nc.sync.dma_start(wd1[:], wd1_d[:])
        wd2 = const.tile([128, KD * VS], F16)
        nc.sync.dma_start(wd2[:], wd2_d[:])
        offs = const.tile([64, NTILES], F32)
        nc.sync.dma_start(offs[:], offs_d[:])
        idn = const.tile([64, 64], F32)
        nc.sync.dma_start(idn[:], id_d[:])
        if has_bd:
            bd = const.tile([64, VS], F32)
            nc.sync.dma_start(bd[:], bd_d[:])

        h_cur = hpool.tile([128, 8 * 64], F32, name="h_sb")
        nc.sync.dma_start(h_cur[:], h0t[:])
        c_cur = cpool.tile([64, 128], F32, name="c_sb")
        nc.sync.dma_start(c_cur[:], c0[:])
        zx_cur = zxpool.tile([64, 512], F32, name="zx_sb")
        nc.sync.dma_start(zx_cur[:], zx0_d[:])

        # h fp16 split tiles for the dense (produced per step, per K-chunk)
        def split_tiles():
            a1 = spool.tile([128, 8 * 64], F16, name="a1")
            a1s = spool.tile([128, 8 * 64], F16, name="a1s")
            a2 = spool.tile([128, 8 * 64], F16, name="a2")
            return a1, a1s, a2

        def emit_split_chunk(h_t, sp, u):
            a1, a1s, a2 = sp
            s = slice(64 * u, 64 * (u + 1))
            nc.vector.tensor_copy(a1[:, s], h_t[:, s])                 # fp16(h)
            nc.vector.tensor_scalar_mul(a1s[:, s], a1[:, s], SC)       # exact
            tmp = gates.tile([128, 64], F32, name="sptmp")
            nc.vector.tensor_sub(tmp[:], h_t[:, s], a1[:, s])
            nc.vector.tensor_scalar_mul(a2[:, s], tmp[:], SC)

        # fp32 self-loading matmuls tolerate only one sync wait; make the PE
        # observe each DMA-loaded tensor it reads via tiny dummy matmuls.
        wps = dpsum.tile([64, NT], F32, name="dps")
        for src in (whh, wd1, wd2, idn, h_cur):
            nc.tensor.matmul(
                wps[0:1, 0:1], lhsT=src[0:32, 0:1], rhs=src[0:32, 0:1],
                start=True, stop=True, skip_group_check=True,
            )

        def emit_z_h(zps, h_t):
            # z_h[64, 512] = h @ W_hh[:, own cols]; fills the PE while the
            # previous step's argmax/AG/gather chain runs on other engines.
            for k in range(8):
                nc.tensor.matmul(
                    zps[:],
                    lhsT=h_t[:, 64 * k : 64 * (k + 1)],
                    rhs=whh[:, 512 * k : 512 * (k + 1)],
                    start=(k == 0),
                    stop=(k == 7),
                )

        zps_cur = zpsum.tile([64, 512], F32, name="zps")
        emit_z_h(zps_cur, h_cur)

        for t in range(T):
            zps = zps_cur
            zx = zx_cur

            # ---- z = z_h + (x @ W_ih + b)  [gathered] ----
            z_sb = gates.tile([64, 512], F32, name="z_sb")
            nc.vector.tensor_add(z_sb[:], zps[:], zx[:])

            # ---- LSTM cell, batch-major [64, 128] (gate cols ordered i,f,o,g;
            # the g-gate weight columns carry a host-side x2 so one tanh
            # scale=0.5 serves all four gates) ----
            tact = gates.tile([64, 512], F32, name="tact")
            nc.scalar.activation(tact[:], z_sb[:], TANH, scale=0.5)
            sig3 = gates.tile([64, 384], F32, name="sig3")
            nc.vector.tensor_scalar(sig3[:], tact[:, 0:384], 0.5, 0.5, OP.mult, OP.add)
            si, sf, so = sig3[:, 0:128], sig3[:, 128:256], sig3[:, 256:384]
            tg = tact[:, 384:512]
            q1 = gates.tile([64, 128], F32, name="q1")
            nc.vector.tensor_mul(q1[:], sf, c_cur[:])
            q2 = gates.tile([64, 128], F32, name="q2")
            nc.vector.tensor_mul(q2[:], si, tg)
            c_new = cpool.tile([64, 128], F32, name="c_sb")
            nc.vector.tensor_add(c_new[:], q1[:], q2[:])
            c_cur = c_new
            tcn = gates.tile([64, 128], F32, name="tcn")
            nc.scalar.activation(tcn[:], c_new[:], TANH)
            hnew = gates.tile([64, 128], F32, name="hnew")
            nc.vector.tensor_mul(hnew[:], so, tcn[:])

            # ---- transpose h slice to [128, 64], AllGather full h.T ----
            tph = tpsum.tile([128, 64], F32, name="tph")
            nc.tensor.transpose(tph[:], hnew[:], idn[:])
            hT = gates.tile([128, 64], F32, name="hT")
            nc.vector.tensor_copy(hT[:], tph[:])
            hsl = dram.tile([128, 64], F32, name="hsl")
            nc.gpsimd.dma_start(hsl[:], hT[:])
            hall = dram.tile([NC * 128, 64], F32, name="hall", addr_space="Shared")
            nc.gpsimd.collective_compute(
                "AllGather",
                OP.bypass,
                replica_groups=RG,
                ins=[hsl[:].opt()],
                outs=[hall[:].opt()],
            )
            h_new_sb = hpool.tile([128, 8 * 64], F32, name="h_sb")
            sp = split_tiles()
            for u in range(8):
                nc.sync.dma_start(
                    h_new_sb[:, 64 * u : 64 * (u + 1)],
                    hall[128 * u : 128 * (u + 1), :],
                )
                emit_split_chunk(h_new_sb, sp, u)
            h_cur = h_new_sb
            a1, a1s, a2 = sp

            # ---- dense: logits = 2^-11 (A1s@W1 + A2@W1 + A1@W2) ----
            logits = lpool.tile([64, VS], F32, name="logits")
            if t < T - 1:
                lmax_all = ampool.tile([64, 8 * NTILES], F32, name="lmax_all")
                lidx_all = ampool.tile([64, 8 * NTILES], U32, name="lidx_all")
            for n in range(NTILES):
                pr = dpsum.tile([64, NT], F32, name="dps")
                for lhs, w, st, sp_ in (
                    (a1s, wd1, True, False),
                    (a2, wd1, False, False),
                    (a1, wd2, False, True),
                ):
                    for k in range(KD):
                        nc.tensor.matmul(
                            pr[:],
                            lhsT=lhs[:, 64 * k : 64 * (k + 1)],
                            rhs=w[:, VS * k + NT * n : VS * k + NT * (n + 1)],
                            start=(st and k == 0),
                            stop=(sp_ and k == KD - 1),
                        )
                if has_bd:
                    tmpl = gates.tile([64, NT], F32, name="tmpl")
                    nc.vector.tensor_scalar_mul(tmpl[:], pr[:], 1.0 / SC)
                    nc.vector.tensor_add(
                        logits[:, NT * n : NT * (n + 1)], tmpl[:], bd[:, NT * n : NT * (n + 1)]
                    )
                else:
                    nc.vector.tensor_scalar_mul(
                        logits[:, NT * n : NT * (n + 1)], pr[:], 1.0 / SC
                    )
                if t < T - 1:
                    # per-group top-8: hides entirely under the dense matmuls
                    nc.vector.max(
                        out=lmax_all[:, 8 * n : 8 * (n + 1)],
                        in_=logits[:, NT * n : NT * (n + 1)],
                    )
                    nc.vector.max_index(
                        lidx_all[:, 8 * n : 8 * (n + 1)],
                        lmax_all[:, 8 * n : 8 * (n + 1)],
                        logits[:, NT * n : NT * (n + 1)],
                    )

            nc.sync.dma_start(out_d[t], logits[:])

            if t == T - 1:
                break

            # next step's h-part matmuls fill the PE during argmax/AG/gather
            zps_cur = zpsum.tile([64, 512], F32, name="zps")
            emit_z_h(zps_cur, h_cur)

            # ---- merge the 8 per-group candidates (first-occurrence ties) ----
            v3d = lmax_all[:].rearrange("b (g j) -> b g j", j=8)
            i3d = lidx_all[:].rearrange("b (g j) -> b g j", j=8)
            vals8 = v3d[:, :, 0]
            pk = ampool.tile([64, 2], F32, name="pk")
            nc.vector.tensor_reduce(
                pk[:, 0:1], vals8, axis=mybir.AxisListType.X, op=OP.max
            )
            gidx8 = ampool.tile([64, NTILES], F32, name="gidx8")
            nc.vector.tensor_tensor(out=gidx8[:], in0=i3d[:, :, 0], in1=offs[:], op=OP.add)
            leq = ampool.tile([64, NTILES], U32, name="leq")
            nc.vector.tensor_tensor(
                out=leq[:], in0=vals8, in1=pk[:, 0:1].to_broadcast([64, NTILES]),
                op=OP.is_equal,
            )
            lpick = ampool.tile([64, NTILES], F32, name="lpick")
            nc.vector.memset(lpick[:], 1.0e9)
            nc.vector.copy_predicated(lpick[:], leq[:], gidx8[:])
            nc.vector.tensor_reduce(
                pk[:, 1:2], lpick[:], axis=mybir.AxisListType.X, op=OP.min
            )

            # ---- global argmax combine via tiny AllGather ----
            amin = dram.tile([64, 2], F32, name="amin")
            nc.gpsimd.dma_start(amin[:], pk[:])
            amout = dram.tile([NC * 64, 2], F32, name="amout", addr_space="Shared")
            nc.gpsimd.collective_compute(
                "AllGather",
                OP.bypass,
                replica_groups=RG,
                ins=[amin[:].opt()],
                outs=[amout[:].opt()],
            )
            cand = ampool.tile([64, 16], F32, name="cand")
            nc.gpsimd.dma_start(
                cand[:].rearrange("b (c j) -> b c j", j=2),
                amout[:].rearrange("(c b) j -> b c j", c=NC),
            )
            c3 = cand[:].rearrange("b (c j) -> b c j", j=2)
            vals = c3[:, :, 0]
            idxs = c3[:, :, 1]
            gmx = ampool.tile([64, 1], F32, name="gmx")
            nc.vector.tensor_reduce(gmx[:], vals, axis=mybir.AxisListType.X, op=OP.max)
            eq = ampool.tile([64, 8], U32, name="eq")
            nc.vector.tensor_tensor(
                out=eq[:], in0=vals, in1=gmx[:].to_broadcast([64, 8]), op=OP.is_equal
            )
            pick = ampool.tile([64, 8], F32, name="pick")
            nc.vector.memset(pick[:], 1.0e9)
            nc.vector.copy_predicated(pick[:], eq[:], idxs)
            gixf = ampool.tile([64, 1], F32, name="gixf")
            nc.vector.tensor_reduce(gixf[:], pick[:], axis=mybir.AxisListType.X, op=OP.min)
            gi32 = ampool.tile([64, 1], I32, name="gi32")
            nc.vector.tensor_copy(gi32[:], gixf[:])

            # ---- gather next step's x-side pre-activations ----
            zx_next = zxpool.tile([64, 512], F32, name="zx_sb")
            nc.gpsimd.indirect_dma_start(
                out=zx_next[:],
                out_offset=None,
                in_=videmb_d[:],
                in_offset=bass.IndirectOffsetOnAxis(ap=gi32[:, :1], axis=0),
            )
            zx_cur = zx_next

    nc.compile()
    return nc


def make_in_maps(inputs: dict, T: int = T_FULL):
    h0 = np.ascontiguousarray(np.asarray(inputs["h0"], np.float32))
    c0 = np.ascontiguousarray(np.asarray(inputs["c0"], np.float32))
    emb = np.ascontiguousarray(np.asarray(inputs["emb"], np.float32))
    W_ih = np.asarray(inputs["W_ih"], np.float32)
    W_hh = np.asarray(inputs["W_hh"], np.float32)
    b = np.asarray(inputs["b"], np.float32)
    W_d = np.asarray(inputs["W_dense"], np.float32)
    b_d = np.asarray(inputs["b_dense"], np.float32)

    has_bd = bool(np.any(b_d != 0))

    h0t = np.ascontiguousarray(
        h0.T.reshape(8, 128, 64).transpose(1, 0, 2).reshape(128, 512)
    )
    ident = np.eye(64, dtype=np.float32)

    # videmb = emb @ W_ih + b in float64, per-core column slice
    emb64 = emb.astype(np.float64)
    Wih64 = W_ih.astype(np.float64)
    b64 = b.astype(np.float64)

    in_maps = []
    for c in range(NC):
        # gate-column order (i, f, o, g); g columns carry x2 so a single
        # tanh(0.5*z) activation serves sigmoid gates and the g tanh alike
        ucols = np.concatenate(
            [np.arange(g * U + 128 * c, g * U + 128 * (c + 1)) for g in (0, 1, 3, 2)]
        )
        gscale = np.ones(512, np.float64)
        gscale[384:] = 2.0
        videmb = ((emb64 @ Wih64[:, ucols] + b64[ucols]) * gscale).astype(np.float32)
        zx0 = np.ascontiguousarray(np.repeat(videmb[GO][None, :], B, axis=0))
        Whh_c = W_hh[:, ucols] * gscale.astype(np.float32)  # [1024, 512]
        whh_l = np.ascontiguousarray(
            Whh_c.reshape(8, 128, 512).transpose(1, 0, 2).reshape(128, 8 * 512)
        )
        Wd_c = W_d[:, VS * c : VS * (c + 1)]  # [1024, 4000]
        W1 = Wd_c.astype(np.float16)
        W2 = ((Wd_c - W1.astype(np.float32)) * SC).astype(np.float16)
        lay16 = lambda M: np.ascontiguousarray(
            M.reshape(KD, 128, VS).transpose(1, 0, 2).reshape(128, KD * VS)
        )
        c0_c = np.ascontiguousarray(c0[:, 128 * c : 128 * (c + 1)])
        offs8 = np.repeat(
            (np.arange(NTILES, dtype=np.float32) * NT + VS * c)[None, :], B, axis=0
        )
        m = {
            "h0t": h0t,
            "c0": c0_c,
            "videmb": videmb,
            "zx0": zx0,
            "whh": whh_l,
            "wd1": lay16(W1),
            "wd2": lay16(W2),
            "offs8": np.ascontiguousarray(offs8),
            "ident": ident,
        }
        if has_bd:
            m["bd"] = np.ascontiguousarray(
                np.repeat(b_d[VS * c : VS * (c + 1)][None, :], B, axis=0)
            )
        in_maps.append(m)
    return in_maps, has_bd, False


def assemble_output(results, T: int = T_FULL):
    parts = [np.asarray(r["out"]).reshape(T, B, VS) for r in results]
    full = np.concatenate(parts, axis=2)  # [T, 64, 32000]
    return np.ascontiguousarray(full.transpose(1, 0, 2))  # [64, T, 32000]


def kernel(**inputs) -> np.ndarray:
    in_maps, has_bd, _ = make_in_maps(inputs)
    nc = build_program(T_FULL, has_bd=has_bd)
    res = run_bass_kernel_spmd(nc, in_maps, core_ids=list(range(NC)))
    return assemble_output(res.results)


if __name__ == "__main__":
    print("kernel module OK")


# revision 20
# speedup vs baseline: 1.0089x; 1.0089x over previous
"""Trainium2 Bass kernel for nn_ChatDecoder: greedy LSTM decoder, 32 steps.

Strategy (8 NeuronCores, SPMD):
  - Vocab-sharded dense projection: each core holds its W_dense[:, c*4000:(c+1)*4000]
    slice resident in SBUF (as an fp16 two-term split: W1=fp16(W),
    W2=fp16((W-W1)*2^11)) and computes logits [64, 4000] per step with
    fp32-quality accuracy at fp16 matmul speed:
      logits = 2^-11 * (A1s@W1 + A2@W1 + A1@W2),  A1=fp16(h), A1s=A1*2^11,
      A2=fp16((h-A1)*2^11) — all three accumulate in one PSUM bank.
  - Unit-sharded LSTM: each core computes 128 of the 1024 hidden units
    (all four gates, batch-major [64, 128] tiles), then an AllGather
    assembles full h.T on every core (8 per-block DMAs back to SBUF).
  - The x @ W_ih + b path is folded into a host-precomputed (float64) table
    videmb = emb @ W_ih[:, own-cols] + b[own-cols]  [32000, 512] per core;
    each step indirect-DMA-gathers videmb[idx] -> zx, so only h @ W_hh runs
    on the PE (8 wide fp32 matmuls, emitted early so they overlap the
    previous step's argmax/AllGather/gather chain).
  - Greedy argmax: two-half DVE max/max_index (first half hidden under the
    dense), tiny AllGather of (value, global index), local combine with
    first-occurrence tie-break matching jnp.argmax.
  - Gates use tanh only: sigmoid(x) = 0.5 + 0.5*tanh(x/2) (tanh's table is
    ~10x more accurate than sigmoid's; greedy decode here has argmax
    margins down to 1.4e-6, so everything on the argmax path is kept at
    fp32-or-better accuracy).

Output per core: [32, 64, 4000] (step, batch, vocab shard); host concatenates
shards and transposes to [64, 32, 32000].
"""

import sys
from contextlib import ExitStack

import numpy as np

for _p in ("/opt/trn_rl_repo",):
    if _p not in sys.path:
        sys.path.insert(0, _p)

import concourse.bass as bass
import concourse.tile as tile
from concourse import bacc, mybir
from concourse.bass_utils import run_bass_kernel_spmd

F32 = mybir.dt.float32
F16 = mybir.dt.float16
I32 = mybir.dt.int32
U32 = mybir.dt.uint32
TANH = mybir.ActivationFunctionType.Tanh
OP = mybir.AluOpType

V, E, U, B, T_FULL = 32000, 512, 1024, 64, 32
NC = 8
VS = V // NC          # 4000 vocab shard
NT = 500              # dense moving tile (<=512)
NTILES = VS // NT     # 8
KD = U // 128         # 8 dense K-chunks
GO = 1
RG = [list(range(NC))]
SC = 2048.0           # 2^11 split scale


def build_program(T: int = T_FULL, has_bd: bool = False):
    nc = bacc.Bacc(
        "TRN2", target_bir_lowering=False, debug=False, num_devices=NC
    )

    def inp(name, shape, dtype=F32):
        return nc.dram_tensor(name, list(shape), dtype, kind="ExternalInput")

    h0t = inp("h0t", (128, 8 * 64))
    c0 = inp("c0", (64, 128))
    videmb_d = inp("videmb", (V, 512))
    zx0_d = inp("zx0", (64, 512))
    whh_d = inp("whh", (128, 8 * 512))
    wd1_d = inp("wd1", (128, KD * VS), F16)
    wd2_d = inp("wd2", (128, KD * VS), F16)
    offs_d = inp("offs8", (64, NTILES))
    id_d = inp("ident", (64, 64))
    if has_bd:
        bd_d = inp("bd", (64, VS))
    out_d = nc.dram_tensor("out", [T, B, VS], F32, kind="ExternalOutput")

    with tile.TileContext(nc) as tc, ExitStack() as ctx:
        const = ctx.enter_context(tc.tile_pool(name="const", bufs=1))
        hpool = ctx.enter_context(tc.tile_pool(name="hpool", bufs=2))
        spool = ctx.enter_context(tc.tile_pool(name="spool", bufs=2))
        cpool = ctx.enter_context(tc.tile_pool(name="cpool", bufs=2))
        zxpool = ctx.enter_context(tc.tile_pool(name="zxpool", bufs=2))
        gates = ctx.enter_context(tc.tile_pool(name="gates", bufs=2))
        lpool = ctx.enter_context(tc.tile_pool(name="lpool", bufs=1))
        ampool = ctx.enter_context(tc.tile_pool(name="ampool", bufs=2))
        dram = ctx.enter_context(tc.tile_pool(name="dram", bufs=2, space="DRAM"))
        zpsum = ctx.enter_context(tc.tile_pool(name="zpsum", bufs=1, space="PSUM"))
        dpsum = ctx.enter_context(tc.tile_pool(name="dpsum", bufs=3, space="PSUM"))
        tpsum = ctx.enter_context(tc.tile_pool(name="tpsum", bufs=2, space="PSUM"))

        whh = const.tile([128, 8 * 512], F32)
        nc.sync.dma_start(whh[:], whh_d[:])
        wd1 = const.tile([128, KD * VS], F16)
        nc.sync.dma_start(wd1[:], wd1_d[:])
        wd2 = const.tile([128, KD * VS], F16)
        nc.sync.dma_start(wd2[:], wd2_d[:])
        offs = const.tile([64, NTILES], F32)
        nc.sync.dma_start(offs[:], offs_d[:])
        idn = const.tile([64, 64], F32)
        nc.sync.dma_start(idn[:], id_d[:])
        if has_bd:
            bd = const.tile([64, VS], F32)
            nc.sync.dma_start(bd[:], bd_d[:])

        h_cur = hpool.tile([128, 8 * 64], F32, name="h_sb")
        nc.sync.dma_start(h_cur[:], h0t[:])
        c_cur = cpool.tile([64, 128], F32, name="c_sb")
        nc.sync.dma_start(c_cur[:], c0[:])
        zx_cur = zxpool.tile([64, 512], F32, name="zx_sb")
        nc.sync.dma_start(zx_cur[:], zx0_d[:])

        # h fp16 split tiles for the dense (produced per step, per K-chunk)
        def split_tiles():
            a1 = spool.tile([128, 8 * 64], F16, name="a1")
            a1s = spool.tile([128, 8 * 64], F16, name="a1s")
            a2 = spool.tile([128, 8 * 64], F16, name="a2")
            return a1, a1s, a2

        def emit_split_chunk(h_t, sp, u):
            a1, a1s, a2 = sp
            s = slice(64 * u, 64 * (u + 1))
            nc.vector.tensor_copy(a1[:, s], h_t[:, s])                 # fp16(h)
            nc.vector.tensor_scalar_mul(a1s[:, s], a1[:, s], SC)       # exact
            tmp = gates.tile([128, 64], F32, name="sptmp")
            nc.vector.tensor_sub(tmp[:], h_t[:, s], a1[:, s])
            nc.vector.tensor_scalar_mul(a2[:, s], tmp[:], SC)

        # fp32 self-loading matmuls tolerate only one sync wait; make the PE
        # observe each DMA-loaded tensor it reads via tiny dummy matmuls.
        wps = dpsum.tile([64, NT], F32, name="dps")
        for src in (whh, wd1, wd2, idn, h_cur):
            nc.tensor.matmul(
                wps[0:1, 0:1], lhsT=src[0:32, 0:1], rhs=src[0:32, 0:1],
                start=True, stop=True, skip_group_check=True,
            )

        def emit_z_h(zps, h_t):
            # z_h[64, 512] = h @ W_hh[:, own cols]; fills the PE while the
            # previous step's argmax/AG/gather chain runs on other engines.
            for k in range(8):
                nc.tensor.matmul(
                    zps[:],
                    lhsT=h_t[:, 64 * k : 64 * (k + 1)],
                    rhs=whh[:, 512 * k : 512 * (k + 1)],
                    start=(k == 0),
                    stop=(k == 7),
                )

        zps_cur = zpsum.tile([64, 512], F32, name="zps")
        emit_z_h(zps_cur, h_cur)

        for t in range(T):
            zps = zps_cur
            zx = zx_cur

            # ---- z = z_h + (x @ W_ih + b)  [gathered] ----
            z_sb = gates.tile([64, 512], F32, name="z_sb")
            nc.vector.tensor_add(z_sb[:], zps[:], zx[:])

            # ---- LSTM cell, batch-major [64, 128] (gate cols ordered i,f,o,g;
            # the g-gate weight columns carry a host-side x2 so one tanh
            # scale=0.5 serves all four gates) ----
            tact = gates.tile([64, 512], F32, name="tact")
            nc.scalar.activation(tact[:], z_sb[:], TANH, scale=0.5)
            sig3 = gates.tile([64, 384], F32, name="sig3")
            nc.vector.tensor_scalar(sig3[:], tact[:, 0:384], 0.5, 0.5, OP.mult, OP.add)
            si, sf, so = sig3[:, 0:128], sig3[:, 128:256], sig3[:, 256:384]
            tg = tact[:, 384:512]
            q1 = gates.tile([64, 128], F32, name="q1")
            nc.vector.tensor_mul(q1[:], sf, c_cur[:])
            q2 = gates.tile([64, 128], F32, name="q2")
            nc.vector.tensor_mul(q2[:], si, tg)
            c_new = cpool.tile([64, 128], F32, name="c_sb")
            nc.vector.tensor_add(c_new[:], q1[:], q2[:])
            c_cur = c_new
            tcn = gates.tile([64, 128], F32, name="tcn")
            nc.scalar.activation(tcn[:], c_new[:], TANH)
            hnew = gates.tile([64, 128], F32, name="hnew")
            nc.vector.tensor_mul(hnew[:], so, tcn[:])

            # ---- transpose h slice to [128, 64], AllGather full h.T ----
            tph = tpsum.tile([128, 64], F32, name="tph")
            nc.tensor.transpose(tph[:], hnew[:], idn[:])
            hT = gates.tile([128, 64], F32, name="hT")
            nc.vector.tensor_copy(hT[:], tph[:])
            hsl = dram.tile([128, 64], F32, name="hsl")
            nc.sync.dma_start(hsl[:], hT[:])
            hall = dram.tile([NC * 128, 64], F32, name="hall", addr_space="Shared")
            nc.gpsimd.collective_compute(
                "AllGather",
                OP.bypass,
                replica_groups=RG,
                ins=[hsl[:].opt()],
                outs=[hall[:].opt()],
            )
            h_new_sb = hpool.tile([128, 8 * 64], F32, name="h_sb")
            sp = split_tiles()
            for u in range(8):
                nc.sync.dma_start(
                    h_new_sb[:, 64 * u : 64 * (u + 1)],
                    hall[128 * u : 128 * (u + 1), :],
                )
                emit_split_chunk(h_new_sb, sp, u)
            h_cur = h_new_sb
            a1, a1s, a2 = sp

            # ---- dense: logits = 2^-11 (A1s@W1 + A2@W1 + A1@W2) ----
            logits = lpool.tile([64, VS], F32, name="logits")
            if t < T - 1:
                lmax_all = ampool.tile([64, 8 * NTILES], F32, name="lmax_all")
                lidx_all = ampool.tile([64, 8 * NTILES], U32, name="lidx_all")
            for n in range(NTILES):
                pr = dpsum.tile([64, NT], F32, name="dps")
                for lhs, w, st, sp_ in (
                    (a1s, wd1, True, False),
                    (a2, wd1, False, False),
                    (a1, wd2, False, True),
                ):
                    for k in range(KD):
                        nc.tensor.matmul(
                            pr[:],
                            lhsT=lhs[:, 64 * k : 64 * (k + 1)],
                            rhs=w[:, VS * k + NT * n : VS * k + NT * (n + 1)],
                            start=(st and k == 0),
                            stop=(sp_ and k == KD - 1),
                        )
                if has_bd:
                    tmpl = gates.tile([64, NT], F32, name="tmpl")
                    nc.vector.tensor_scalar_mul(tmpl[:], pr[:], 1.0 / SC)
                    nc.vector.tensor_add(
                        logits[:, NT * n : NT * (n + 1)], tmpl[:], bd[:, NT * n : NT * (n + 1)]
                    )
                else:
                    nc.vector.tensor_scalar_mul(
                        logits[:, NT * n : NT * (n + 1)], pr[:], 1.0 / SC
                    )
                if t < T - 1:
                    # per-group top-8: hides entirely under the dense matmuls
                    nc.vector.max(
                        out=lmax_all[:, 8 * n : 8 * (n + 1)],
                        in_=logits[:, NT * n : NT * (n + 1)],
                    )
                    nc.vector.max_index(
                        lidx_all[:, 8 * n : 8 * (n + 1)],
                        lmax_all[:, 8 * n : 8 * (n + 1)],
                        logits[:, NT * n : NT * (n + 1)],
                    )

            nc.sync.dma_start(out_d[t], logits[:])

            if t == T - 1:
                break

            # next step's h-part matmuls fill the PE during argmax/AG/gather
            zps_cur = zpsum.tile([64, 512], F32, name="zps")
            emit_z_h(zps_cur, h_cur)

            # ---- merge the 8 per-group candidates (first-occurrence ties) ----
            v3d = lmax_all[:].rearrange("b (g j) -> b g j", j=8)
            i3d = lidx_all[:].rearrange("b (g j) -> b g j", j=8)
            vals8 = v3d[:, :, 0]
            pk = ampool.tile([64, 2], F32, name="pk")
            nc.vector.tensor_reduce(
                pk[:, 0:1], vals8, axis=mybir.AxisListType.X, op=OP.max
            )
            gidx8 = ampool.tile([64, NTILES], F32, name="gidx8")
            nc.vector.tensor_tensor(out=gidx8[:], in0=i3d[:, :, 0], in1=offs[:], op=OP.add)
            leq = ampool.tile([64, NTILES], U32, name="leq")
            nc.vector.tensor_tensor(
                out=leq[:], in0=vals8, in1=pk[:, 0:1].to_broadcast([64, NTILES]),
                op=OP.is_equal,
            )
            lpick = ampool.tile([64, NTILES], F32, name="lpick")
            nc.vector.memset(lpick[:], 1.0e9)
            nc.vector.copy_predicated(lpick[:], leq[:], gidx8[:])
            nc.vector.tensor_reduce(
                pk[:, 1:2], lpick[:], axis=mybir.AxisListType.X, op=OP.min
            )

            # ---- global argmax combine via tiny AllGather ----
            amin = dram.tile([64, 2], F32, name="amin")
            nc.sync.dma_start(amin[:], pk[:])
            amout = dram.tile([NC * 64, 2], F32, name="amout", addr_space="Shared")
            nc.gpsimd.collective_compute(
                "AllGather",
                OP.bypass,
                replica_groups=RG,
                ins=[amin[:].opt()],
                outs=[amout[:].opt()],
            )
            cand = ampool.tile([64, 16], F32, name="cand")
            nc.sync.dma_start(
                cand[:].rearrange("b (c j) -> b c j", j=2),
                amout[:].rearrange("(c b) j -> b c j", c=NC),
            )
            c3 = cand[:].rearrange("b (c j) -> b c j", j=2)
            vals = c3[:, :, 0]
            idxs = c3[:, :, 1]
            gmx = ampool.tile([64, 1], F32, name="gmx")
            nc.vector.tensor_reduce(gmx[:], vals, axis=mybir.AxisListType.X, op=OP.max)
            eq = ampool.tile([64, 8], U32, name="eq")
            nc.vector.tensor_tensor(
                out=eq[:], in0=vals, in1=gmx[:].to_broadcast([64, 8]), op=OP.is_equal
            )
            pick = ampool.tile([64, 8], F32, name="pick")
            nc.vector.memset(pick[:], 1.0e9)
            nc.vector.copy_predicated(pick[:], eq[:], idxs)
            gixf = ampool.tile([64, 1], F32, name="gixf")
            nc.vector.tensor_reduce(gixf[:], pick[:], axis=mybir.AxisListType.X, op=OP.min)
            gi32 = ampool.tile([64, 1], I32, name="gi32")
            nc.vector.tensor_copy(gi32[:], gixf[:])

            # ---- gather next step's x-side pre-activations ----
            zx_next = zxpool.tile([64, 512], F32, name="zx_sb")
            nc.gpsimd.indirect_dma_start(
                out=zx_next[:],
                out_offset=None,
                in_=videmb_d[:],
                in_offset=bass.IndirectOffsetOnAxis(ap=gi32[:, :1], axis=0),
            )
            zx_cur = zx_next

    nc.compile()
    return nc


def make_in_maps(inputs: dict, T: int = T_FULL):
    h0 = np.ascontiguousarray(np.asarray(inputs["h0"], np.float32))
    c0 = np.ascontiguousarray(np.asarray(inputs["c0"], np.float32))
    emb = np.ascontiguousarray(np.asarray(inputs["emb"], np.float32))
    W_ih = np.asarray(inputs["W_ih"], np.float32)
    W_hh = np.asarray(inputs["W_hh"], np.float32)
    b = np.asarray(inputs["b"], np.float32)
    W_d = np.asarray(inputs["W_dense"], np.float32)
    b_d = np.asarray(inputs["b_dense"], np.float32)

    has_bd = bool(np.any(b_d != 0))

    h0t = np.ascontiguousarray(
        h0.T.reshape(8, 128, 64).transpose(1, 0, 2).reshape(128, 512)
    )
    ident = np.eye(64, dtype=np.float32)

    # videmb = emb @ W_ih + b in float64, per-core column slice
    emb64 = emb.astype(np.float64)
    Wih64 = W_ih.astype(np.float64)
    b64 = b.astype(np.float64)

    in_maps = []
    for c in range(NC):
        # gate-column order (i, f, o, g); g columns carry x2 so a single
        # tanh(0.5*z) activation serves sigmoid gates and the g tanh alike
        ucols = np.concatenate(
            [np.arange(g * U + 128 * c, g * U + 128 * (c + 1)) for g in (0, 1, 3, 2)]
        )
        gscale = np.ones(512, np.float64)
        gscale[384:] = 2.0
        videmb = ((emb64 @ Wih64[:, ucols] + b64[ucols]) * gscale).astype(np.float32)
        zx0 = np.ascontiguousarray(np.repeat(videmb[GO][None, :], B, axis=0))
        Whh_c = W_hh[:, ucols] * gscale.astype(np.float32)  # [1024, 512]
        whh_l = np.ascontiguousarray(
            Whh_c.reshape(8, 128, 512).transpose(1, 0, 2).reshape(128, 8 * 512)
        )
        Wd_c = W_d[:, VS * c : VS * (c + 1)]  # [1024, 4000]
        W1 = Wd_c.astype(np.float16)
        W2 = ((Wd_c - W1.astype(np.float32)) * SC).astype(np.float16)
        lay16 = lambda M: np.ascontiguousarray(
            M.reshape(KD, 128, VS).transpose(1, 0, 2).reshape(128, KD * VS)
        )
        c0_c = np.ascontiguousarray(c0[:, 128 * c : 128 * (c + 1)])
        offs8 = np.repeat(
            (np.arange(NTILES, dtype=np.float32) * NT + VS * c)[None, :], B, axis=0
        )
        m = {
            "h0t": h0t,
            "c0": c0_c,
            "videmb": videmb,
            "zx0": zx0,
            "whh": whh_l,
            "wd1": lay16(W1),
            "wd2": lay16(W2),
            "offs8": np.ascontiguousarray(offs8),
            "ident": ident,
        }
        if has_bd:
            m["bd"] = np.ascontiguousarray(
                np.repeat(b_d[VS * c : VS * (c + 1)][None, :], B, axis=0)
            )
        in_maps.append(m)
    return in_maps, has_bd, False


def assemble_output(results, T: int = T_FULL):
    parts = [np.asarray(r["out"]).reshape(T, B, VS) for r in results]
    full = np.concatenate(parts, axis=2)  # [T, 64, 32000]
    return np.ascontiguousarray(full.transpose(1, 0, 2))  # [64, T, 32000]


def kernel(**inputs) -> np.ndarray:
    in_maps, has_bd, _ = make_in_maps(inputs)
    nc = build_program(T_FULL, has_bd=has_bd)
    res = run_bass_kernel_spmd(nc, in_maps, core_ids=list(range(NC)))
    return assemble_output(res.results)


if __name__ == "__main__":
    print("kernel module OK")
